# revision 1
# baseline (speedup 1.0000x reference)
"""Kernel builder for the dual-stream linear-attention transformer (per-core).

Layout convention:
  - "layout 1" activation: [E, N] feature-major; SBUF tiles [128, KE, C]
    (feature f = 128*k + p -> partition p, k-th slice; tokens on free dim).
  - "layout 2" activation: [N, E] token-major; SBUF tiles [128(tokens), E].
  - Residual streams live in internal DRAM as [E, N] (layout 1), streamed
    through SBUF in token chunks of C.

fp32r discipline (cfg.f32r): every matmul operand tile is declared
float32r. Producers are either DMA byte-casts (bitcast both sides) or DVE
ops (which round correctly on write). ACT must never WRITE an f32r tile
(hardware produces garbage); ACT/DVE readers view f32r tiles via
.bitcast(F32), which is exact.
"""

from dataclasses import dataclass
from contextlib import ExitStack

import numpy as np

import concourse.bass as bass
import concourse.mybir as mybir
import concourse.tile as tile

F32 = mybir.dt.float32
F32R = mybir.dt.float32r
AF = mybir.ActivationFunctionType
ALU = mybir.AluOpType

LN_EPS = 1e-5
BN_EPS = 1e-5


@dataclass
class Cfg:
    N: int = 2048
    E: int = 512
    R: int = 256
    X: int = 1024
    H: int = 8
    L: int = 3
    OUT: int = 15
    C: int = 512          # token chunk (free dim of layout-1 tiles)
    f32r: bool = True

    @property
    def KE(self):
        return self.E // 128

    @property
    def KR(self):
        return self.R // 128

    @property
    def KX(self):
        return self.X // 128

    @property
    def NC(self):
        return self.N // self.C

    @property
    def NTT(self):
        return self.C // 128  # token tiles per chunk


def host_constants(cfg):
    """Constant tensors passed as extra inputs (identical on every core)."""
    E, H = cfg.E, cfg.H
    dh = E // H
    ident = np.eye(128, dtype=np.float32)
    ones = np.ones((128, 128), dtype=np.float32)
    hmask = np.zeros((E, H), dtype=np.float32)
    for f in range(E):
        hmask[f, f // dh] = 1.0
    cmask = hmask.T.copy()
    return {"ident": ident, "ones128": ones, "hmask": hmask, "cmask": cmask}


PHASES = []


def build(nc, cfg):
    """Declare IO and build the whole program inside a TileContext."""
    c = cfg
    E, R, X, H, N, C, L = c.E, c.R, c.X, c.H, c.N, c.C, c.L
    KE, KR, KX, NC, NTT = c.KE, c.KR, c.KX, c.NC, c.NTT
    E4, E2, E8 = E // 4, E // 2, E // 8

    dt = F32
    MMDT = F32R if c.f32r else F32
    din = {}

    def inp(name, shape):
        din[name] = nc.dram_tensor(name, list(shape), dt, kind="ExternalInput")
        return din[name].ap()

    body_feats = inp("body_feats", (N, E))
    limb_feats = inp("limb_feats", (N, E))
    dw = inp("dw", (L, 4, 3, E, R))
    uw = inp("uw", (L, 4, 3, R, E))
    ub = inp("ub", (L, 4, 3, E))
    ow = inp("ow", (L, 4, E, E))
    ob = inp("ob", (L, 4, E))
    w1 = inp("w1", (L, 2, E, X))
    b1 = inp("b1", (L, 2, X))
    cw = inp("cw", (L, 2, X, 3))
    cb = inp("cb", (L, 2, X))
    bng = inp("bng", (L, 2, X))
    bnb = inp("bnb", (L, 2, X))
    w2 = inp("w2", (L, 2, X, E))
    b2 = inp("b2", (L, 2, E))
    lng = inp("lng", (L, 5, E))
    lnb = inp("lnb", (L, 5, E))
    gw1 = inp("gw1", (L, 2 * E, E4))
    gb1 = inp("gb1", (L, E4))
    gw2 = inp("gw2", (L, E4, 2))
    gb2 = inp("gb2", (L, 2))
    fw1 = inp("fw1", (2 * E, E2))
    fb1 = inp("fb1", (E2,))
    fw2 = inp("fw2", (E2, E))
    fb2 = inp("fb2", (E,))
    flng = inp("flng", (E,))
    flnb = inp("flnb", (E,))
    rw1 = inp("rw1", (E, E4))
    rb1 = inp("rb1", (E4,))
    rw2 = inp("rw2", (E4, E8))
    rb2 = inp("rb2", (E8,))
    rw3 = inp("rw3", (E8, c.OUT))
    rb3 = inp("rb3", (c.OUT,))
    ident_in = inp("ident", (128, 128))
    ones_in = inp("ones128", (128, 128))
    hmask_in = inp("hmask", (E, H))
    cmask_in = inp("cmask", (H, E))

    out_dram = nc.dram_tensor("out", [N, c.OUT], dt, kind="ExternalOutput")

    def idram(name):
        return nc.dram_tensor(name, [E, N], dt).ap().rearrange(
            "(k p) n -> p k n", p=128)

    rs = {}
    for s in ("b", "l"):
        rs[s, 0] = idram(f"r{s}0")
        for l in range(L):
            for st in (1, 2, 3):
                rs[s, (l, st)] = idram(f"r{s}_{l}_{st}")

    lowp = nc.allow_low_precision("f32r matmul operand rounding")

    with tile.TileContext(nc) as tc, ExitStack() as ctx, lowp:
        p_ = ctx.enter_context
        cst = p_(tc.tile_pool(name="cst", bufs=1))
        wbig = p_(tc.tile_pool(name="wbig", bufs=3))
        wsm = p_(tc.tile_pool(name="wsm", bufs=2))
        wcol = p_(tc.tile_pool(name="wcol", bufs=10))
        wrow = p_(tc.tile_pool(name="wrow", bufs=3))
        pa = p_(tc.tile_pool(name="pa", bufs=7))      # 8KB tiles
        pb = p_(tc.tile_pool(name="pb", bufs=3))      # 16KB tiles
        pc = p_(tc.tile_pool(name="pc", bufs=5))      # 2KB tiles
        pat = p_(tc.tile_pool(name="pat", bufs=2))    # per-attn persistents
        phl = p_(tc.tile_pool(name="phl", bufs=3))    # conv halos
        ps = p_(tc.tile_pool(name="ps", bufs=5, space="PSUM"))
        ps1 = p_(tc.tile_pool(name="ps1", bufs=2, space="PSUM"))
        psb = p_(tc.tile_pool(name="psb", bufs=1, space="PSUM"))

        v, sc, gp = nc.vector, nc.scalar, nc.gpsimd

        def mm(out, lhsT, rhs, start, stop):
            nc.tensor.matmul(out, lhsT, rhs, start=start, stop=stop)

        def F(ap):
            """fp32 view of an f32r tile (exact)."""
            return ap.bitcast(F32) if c.f32r else ap

        def M(ap):
            """f32r byte-view of an fp32 DRAM AP (for DMA byte-casts)."""
            return ap.bitcast(MMDT) if c.f32r else ap

        # ---- constants ----
        ident_t = cst.tile([128, 128], dt, tag="ident")
        nc.sync.dma_start(out=ident_t, in_=ident_in)
        ones_t = cst.tile([128, 128], MMDT, tag="ones")
        nc.sync.dma_start(out=ones_t, in_=M(ones_in))
        hmask_t = cst.tile([128, KE, H], dt, tag="hmask")
        nc.sync.dma_start(out=hmask_t,
                          in_=hmask_in.rearrange("(k p) h -> p k h", p=128))
        cmask_t = cst.tile([H, KE, 128], MMDT, tag="cmask")
        nc.sync.dma_start(out=cmask_t,
                          in_=M(cmask_in.rearrange("h (k p) -> h k p", p=128)))
        ONES_COL = ones_t[:, 0:1]
        ONES_ROW = ones_t[0:1, :]
        eps_den = cst.tile([8, 1], dt, tag="epsd")
        v.memset(eps_den, 1e-6)
        eps_ln = cst.tile([1, 1], dt, tag="epsl")
        v.memset(eps_ln, LN_EPS)

        def col_tile(src_ap, m, tag="col"):
            t = wcol.tile([128, m], dt, tag=tag)
            nc.sync.dma_start(out=t, in_=src_ap.rearrange("(m p) -> p m", p=128))
            return t

        def ln_stats_apply(xs, g_col, b_col, out_tiles, relu=False,
                           out_f32r=False):
            """LayerNorm over features (layout 1). xs: KE f32r APs [128, C]."""
            sq = pa.tile([128, KE, C], MMDT, tag="a8")
            for m in range(KE):
                v.tensor_tensor(out=sq[:, m, :], in0=F(xs[m]), in1=F(xs[m]),
                                op=ALU.mult)
            ps_s = ps1.tile([1, C], F32, tag="st")
            ps_ss = ps1.tile([1, C], F32, tag="st")
            for m in range(KE):
                mm(ps_s, ONES_COL, xs[m], start=(m == 0), stop=(m == KE - 1))
                mm(ps_ss, ONES_COL, sq[:, m, :], start=(m == 0),
                   stop=(m == KE - 1))
            arow = pc.tile([1, C], dt, tag="a2")   # mean
            brow = pc.tile([1, C], dt, tag="a2")   # msq -> var -> sd
            trow2 = pc.tile([1, C], dt, tag="a2")  # mean^2
            sc.activation(arow, ps_s, AF.Copy, scale=1.0 / E)
            sc.activation(brow, ps_ss, AF.Copy, scale=1.0 / E)
            sc.activation(trow2, arow, AF.Square)
            v.tensor_tensor(out=brow, in0=brow, in1=trow2, op=ALU.subtract)
            sc.activation(brow, brow, AF.Sqrt, bias=eps_ln[0:1, 0:1])
            srow = pc.tile([1, C], MMDT, tag="a2")
            v.reciprocal(out=srow, in_=brow)
            trow = pc.tile([1, C], MMDT, tag="a2")
            v.tensor_tensor(out=trow, in0=arow, in1=F(srow), op=ALU.mult)
            bc_s = psb.tile([128, C], F32, tag="bc")
            mm(bc_s, ONES_ROW, srow, start=True, stop=True)
            sb_s = pc.tile([128, C], dt, tag="a2")
            sc.activation(sb_s, bc_s, AF.Copy)
            bc_t = psb.tile([128, C], F32, tag="bc")
            mm(bc_t, ONES_ROW, trow, start=True, stop=True)
            sb_t = pc.tile([128, C], dt, tag="a2")
            sc.activation(sb_t, bc_t, AF.Copy)
            tmp = pa.tile([128, KE, C], dt, tag="a8")
            for m in range(KE):
                gp.tensor_tensor(out=tmp[:, m, :], in0=F(xs[m]), in1=sb_s,
                                 op=ALU.mult)
                gp.tensor_tensor(out=tmp[:, m, :], in0=tmp[:, m, :], in1=sb_t,
                                 op=ALU.subtract)
                if not out_f32r:
                    fn = AF.Relu if relu else AF.Identity
                    sc.activation(out_tiles[m], tmp[:, m, :], fn,
                                  bias=b_col[:, m:m + 1],
                                  scale=g_col[:, m:m + 1])
                elif relu:
                    tmpf = pc.tile([128, C], dt, tag="a2")
                    v.tensor_scalar(out=tmpf, in0=tmp[:, m, :],
                                    scalar1=g_col[:, m:m + 1],
                                    scalar2=b_col[:, m:m + 1],
                                    op0=ALU.mult, op1=ALU.add)
                    v.tensor_scalar_max(out_tiles[m], tmpf, 0.0)
                else:
                    v.tensor_scalar(out=out_tiles[m], in0=tmp[:, m, :],
                                    scalar1=g_col[:, m:m + 1],
                                    scalar2=b_col[:, m:m + 1],
                                    op0=ALU.mult, op1=ALU.add)

        def load_x_chunk(dram_l1, ci, tag="a8"):
            xt = pa.tile([128, KE, C], MMDT, tag=tag)
            nc.sync.dma_start(out=xt,
                              in_=M(dram_l1[:, :, ci * C:(ci + 1) * C]))
            return xt

        def store_chunk(dram_l1, ci, t):
            # stores ride the GPSIMD SWDGE queue so they never delay the
            # SP-queue loads that sit on the next phase's critical path
            gp.dma_start(out=dram_l1[:, :, ci * C:(ci + 1) * C], in_=t)

        # ---- entry transpose ----
        def entry(x_ap, dst):
            for ttk in range(N // 128):
                x2 = pa.tile([128, E], dt, tag="a8")
                nc.sync.dma_start(out=x2, in_=x_ap[ttk * 128:(ttk + 1) * 128, :])
                xt = pa.tile([128, KE, 128], dt, tag="a8")
                for f in range(KE):
                    pt = ps.tile([128, 128], F32, tag="mm")
                    nc.tensor.transpose(pt, x2[:, f * 128:(f + 1) * 128],
                                        ident_t)
                    sc.activation(xt[:, f, :], pt, AF.Copy)
                nc.sync.dma_start(out=dst[:, :, ttk * 128:(ttk + 1) * 128],
                                  in_=xt)

        PHASES.append(("entry", len(nc.inst_map)))
        entry(body_feats, rs["b", 0])
        entry(limb_feats, rs["l", 0])

        # ---- linear attention ----
        def attn(l, a, xq_dram, xkv_dram, tail):
            """tail(ci, proj_tiles(f32r, ob added), xq(f32r))."""
            dwt = wbig.tile([128, KE, 3, R], MMDT, tag="w")
            for t3 in range(3):
                nc.sync.dma_start(
                    out=dwt[:, :, t3, :],
                    in_=M(dw[l, a, t3].rearrange("(k p) r -> p k r", p=128)))
            uwt = wbig.tile([128, KR, 3, E], MMDT, tag="w")
            for t3 in range(3):
                nc.sync.dma_start(
                    out=uwt[:, :, t3, :],
                    in_=M(uw[l, a, t3].rearrange("(k p) e -> p k e", p=128)))
            owt = wbig.tile([128, KE, E], MMDT, tag="w")
            nc.sync.dma_start(
                out=owt, in_=M(ow[l, a].rearrange("(k p) e -> p k e", p=128)))
            ubq_col = col_tile(ub[l, a, 0], KE)
            ubk_row = wrow.tile([1, E], MMDT, tag="row")
            nc.sync.dma_start(out=ubk_row, in_=M(ub[l, a, 1][None, :]))
            ubv_row = wrow.tile([1, E], MMDT, tag="row")
            nc.sync.dma_start(out=ubv_row, in_=M(ub[l, a, 2][None, :]))
            ob_col = col_tile(ob[l, a], KE)

            PHASES.append((f"attn{l}.{a}.alpha", len(nc.inst_map)))
            kv_acc = pat.tile([128, 4, 258], dt, tag="kva")

            # alpha: k/v -> kv, ksum (ones column appended to v)
            for ci in range(NC):
                xt = load_x_chunk(xkv_dram, ci)
                lowk = pa.tile([128, KR, C], MMDT, tag="a8")
                lowv = pa.tile([128, KR, C], MMDT, tag="a8")
                for t, low in ((1, lowk), (2, lowv)):
                    pls = [ps.tile([128, C], F32, tag="mm", name=f"pl{_i}")
                           for _i in range(KR)]
                    for k in range(KE):
                        for m in range(KR):
                            mm(pls[m], dwt[:, k, t, m * 128:(m + 1) * 128],
                               xt[:, k, :], start=(k == 0), stop=(k == KE - 1))
                    for m in range(KR):
                        v.tensor_copy(low[:, m, :], pls[m])
                k2f = pa.tile([128, NTT, E], MMDT, tag="a8")
                v2x = pa.tile([128, NTT, 2, 258], MMDT, tag="a8")
                v.memset(F(v2x[:, :, :, 256:258]), 1.0)
                for tt in range(NTT):
                    pk = ps.tile([128, E], F32, tag="mm")
                    pv = ps.tile([128, E], F32, tag="mm")
                    for k in range(KR):
                        mm(pk, lowk[:, k, tt * 128:(tt + 1) * 128],
                           uwt[:, k, 1, :], start=(k == 0), stop=False)
                        mm(pv, lowv[:, k, tt * 128:(tt + 1) * 128],
                           uwt[:, k, 2, :], start=(k == 0), stop=False)
                    mm(pk, ONES_ROW, ubk_row, start=False, stop=True)
                    mm(pv, ONES_ROW, ubv_row, start=False, stop=True)
                    ee = pc.tile([128, E], dt, tag="a2")
                    rr = pc.tile([128, E], dt, tag="a2")
                    sc.activation(ee, pk, AF.Exp)
                    sc.activation(rr, pk, AF.Relu)
                    gp.tensor_scalar_min(ee, ee, 1.0)
                    v.tensor_tensor(out=k2f[:, tt, :], in0=ee, in1=rr,
                                    op=ALU.add)
                    v.tensor_copy(v2x[:, tt, 0, 0:256], pv[:, 0:256])
                    v.tensor_copy(v2x[:, tt, 1, 0:256], pv[:, 256:512])
                pkvs = [ps.tile([128, 258], F32, tag="mm", name=f"pkv{_i}")
                        for _i in range(4)]
                for tt in range(NTT):
                    for p in range(4):
                        mm(pkvs[p], k2f[:, tt, p * 128:(p + 1) * 128],
                           v2x[:, tt, p // 2, :],
                           start=(tt == 0), stop=(tt == NTT - 1))
                for p in range(4):
                    if ci == 0:
                        sc.activation(kv_acc[:, p, :], pkvs[p], AF.Copy)
                    else:
                        v.tensor_tensor(out=kv_acc[:, p, :],
                                        in0=kv_acc[:, p, :], in1=pkvs[p],
                                        op=ALU.add)

            bd = pat.tile([128, KE, 128], MMDT, tag="bd")
            v.memset(F(bd), 0.0)
            for p in range(4):
                h0c = (2 * p % 4) * 64
                h1c = ((2 * p + 1) % 4) * 64
                v.tensor_copy(bd[0:64, p, 0:64], kv_acc[0:64, p, h0c:h0c + 64])
                v.tensor_copy(bd[64:128, p, 64:128],
                              kv_acc[64:128, p, h1c:h1c + 64])
            kmm = pat.tile([128, KE, H], MMDT, tag="km")
            for k in range(KE):
                v.tensor_scalar_mul(kmm[:, k, :], hmask_t[:, k, :],
                                    kv_acc[:, k, 256:257])

            # beta: q -> attention out-proj
            PHASES.append((f"attn{l}.{a}.beta", len(nc.inst_map)))
            for ci in range(NC):
                xq = load_x_chunk(xq_dram, ci)
                lowq = pa.tile([128, KR, C], MMDT, tag="a8")
                pls = [ps.tile([128, C], F32, tag="mm", name=f"plq{_i}") for _i in range(KR)]
                for k in range(KE):
                    for m in range(KR):
                        mm(pls[m], dwt[:, k, 0, m * 128:(m + 1) * 128],
                           xq[:, k, :], start=(k == 0), stop=(k == KE - 1))
                for m in range(KR):
                    v.tensor_copy(lowq[:, m, :], pls[m])
                qf = pa.tile([128, KE, C], MMDT, tag="a8")
                pqs = [ps.tile([128, C], F32, tag="mm", name=f"pq{_i}") for _i in range(KE)]
                for k in range(KR):
                    for m in range(KE):
                        mm(pqs[m], uwt[:, k, 0, m * 128:(m + 1) * 128],
                           lowq[:, k, :], start=(k == 0), stop=(k == KR - 1))
                for m in range(KE):
                    ee = pc.tile([128, C], dt, tag="a2")
                    rr = pc.tile([128, C], dt, tag="a2")
                    sc.activation(ee, pqs[m], AF.Exp, bias=ubq_col[:, m:m + 1])
                    sc.activation(rr, pqs[m], AF.Relu, bias=ubq_col[:, m:m + 1])
                    gp.tensor_scalar_min(ee, ee, 1.0)
                    v.tensor_tensor(out=qf[:, m, :], in0=ee, in1=rr, op=ALU.add)
                pd = ps.tile([8, C], F32, tag="mm")
                for k in range(KE):
                    mm(pd, kmm[:, k, :], qf[:, k, :], start=(k == 0),
                       stop=(k == KE - 1))
                den = pc.tile([8, C], dt, tag="a2")
                sc.activation(den, pd, AF.Identity, bias=eps_den)
                rec = pc.tile([8, C], MMDT, tag="a2")
                v.reciprocal(out=rec, in_=den)
                att = pa.tile([128, KE, C], MMDT, tag="a8")
                for m in range(KE):
                    pn = ps.tile([128, C], F32, tag="mm")
                    mm(pn, bd[:, m, :], qf[:, m, :], start=True, stop=True)
                    pr = ps.tile([128, C], F32, tag="mm")
                    mm(pr, cmask_t[:, m, :], rec, start=True, stop=True)
                    rb = pc.tile([128, C], dt, tag="a2")
                    sc.activation(rb, pr, AF.Copy)
                    v.tensor_tensor(out=att[:, m, :], in0=pn, in1=rb,
                                    op=ALU.mult)
                proj = pa.tile([128, KE, C], MMDT, tag="a8")
                pos = [ps.tile([128, C], F32, tag="mm", name=f"po{_i}") for _i in range(KE)]
                for k in range(KE):
                    for m in range(KE):
                        mm(pos[m], owt[:, k, m * 128:(m + 1) * 128],
                           att[:, k, :], start=(k == 0), stop=(k == KE - 1))
                for m in range(KE):
                    v.tensor_scalar_add(proj[:, m, :], pos[m],
                                        ob_col[:, m:m + 1])
                tail(ci, proj, xq)

        # ---- tails ----
        def make_self_tail(l, s, dst):
            g_col = col_tile(lng[l, 0 if s == "b" else 1], KE, tag="lncol")
            b_col = col_tile(lnb[l, 0 if s == "b" else 1], KE, tag="lncol")

            def tail(ci, proj, xq):
                for m in range(KE):
                    v.tensor_tensor(out=proj[:, m, :], in0=F(proj[:, m, :]),
                                    in1=F(xq[:, m, :]), op=ALU.add)
                outt = pa.tile([128, KE, C], dt, tag="a8")
                ln_stats_apply([proj[:, m, :] for m in range(KE)], g_col, b_col,
                               [outt[:, m, :] for m in range(KE)])
                store_chunk(dst, ci, outt)

            return tail

        def make_cross_tail(l, s, dst):
            gw1t = wsm.tile([128, 2 * KE, E4], MMDT, tag="ws")
            nc.sync.dma_start(out=gw1t,
                              in_=M(gw1[l].rearrange("(k p) g -> p k g", p=128)))
            gw2t = wsm.tile([128, 2], dt, tag="ws")
            nc.sync.dma_start(out=gw2t, in_=gw2[l])
            gwd = pat.tile([128, 1], MMDT, tag="gwd")
            v.tensor_tensor(out=gwd, in0=gw2t[:, 0:1], in1=gw2t[:, 1:2],
                            op=ALU.subtract)
            gb1_col = col_tile(gb1[l], 1, tag="lncol")
            gb2a = pat.tile([1, 1], dt, tag="gb2")
            nc.sync.dma_start(out=gb2a, in_=gb2[l, 0:1][None, :])
            gb2b = pat.tile([1, 1], dt, tag="gb2b")
            nc.sync.dma_start(out=gb2b, in_=gb2[l, 1:2][None, :])
            gb2d = pat.tile([1, 1], dt, tag="gb2d")
            v.tensor_tensor(out=gb2d, in0=gb2a, in1=gb2b, op=ALU.subtract)
            g_col = col_tile(lng[l, 2], KE, tag="lncol")
            b_col = col_tile(lnb[l, 2], KE, tag="lncol")

            def tail(ci, proj, xq):
                pg = ps.tile([128, C], F32, tag="mm")
                for k in range(2 * KE):
                    rhs = xq[:, k, :] if k < KE else proj[:, k - KE, :]
                    mm(pg, gw1t[:, k, :], rhs, start=(k == 0),
                       stop=(k == 2 * KE - 1))
                g1f = pc.tile([128, C], dt, tag="a2")
                v.tensor_scalar(out=g1f, in0=pg, scalar1=gb1_col[:, 0:1],
                                scalar2=0.0, op0=ALU.add, op1=ALU.max)
                g1t = pc.tile([128, C], MMDT, tag="a2")
                v.tensor_scalar_min(g1t, g1f, 6.0)
                pg2 = ps.tile([1, C], F32, tag="mm")
                mm(pg2, gwd, g1t, start=True, stop=True)
                bgf = pc.tile([1, C], dt, tag="a2")
                sc.activation(bgf, pg2, AF.Sigmoid, bias=gb2d[0:1, 0:1])
                bg = pc.tile([1, C], MMDT, tag="a2")
                v.tensor_copy(bg, bgf)
                pbg = psb.tile([128, C], F32, tag="bc")
                mm(pbg, ONES_ROW, bg, start=True, stop=True)
                mt = pa.tile([128, KE, C], MMDT, tag="a8")
                for m in range(KE):
                    dtmp = pc.tile([128, C], dt, tag="a2")
                    gp.tensor_tensor(out=dtmp, in0=F(xq[:, m, :]),
                                     in1=F(proj[:, m, :]), op=ALU.subtract)
                    v.tensor_tensor(out=dtmp, in0=dtmp, in1=pbg, op=ALU.mult)
                    v.tensor_tensor(out=mt[:, m, :], in0=dtmp,
                                    in1=F(proj[:, m, :]), op=ALU.add)
                outt = pa.tile([128, KE, C], dt, tag="a8")
                ln_stats_apply([mt[:, m, :] for m in range(KE)], g_col, b_col,
                               [outt[:, m, :] for m in range(KE)])
                store_chunk(dst, ci, outt)

            return tail

        # ---- FFN ----
        def ffn(l, s, src, dst):
            PHASES.append((f"ffn{l}.{s}", len(nc.inst_map)))
            si = 0 if s == "b" else 1
            w1t = wbig.tile([128, KE, X], MMDT, tag="w")
            nc.sync.dma_start(
                out=w1t, in_=M(w1[l, si].rearrange("(k p) x -> p k x", p=128)))
            w2t = wbig.tile([128, KX, E], MMDT, tag="w")
            nc.sync.dma_start(
                out=w2t, in_=M(w2[l, si].rearrange("(k p) e -> p k e", p=128)))
            b1_col = col_tile(b1[l, si], KX, tag="ffcol")
            b2_col = col_tile(b2[l, si], KE, tag="ffcol")
            w0_col = col_tile(cw[l, si, :, 0], KX, tag="ffcol")
            w1c_col = col_tile(cw[l, si, :, 1], KX, tag="ffcol")
            w2_col = col_tile(cw[l, si, :, 2], KX, tag="ffcol")
            cb_col = col_tile(cb[l, si], KX, tag="ffcol")
            bng_col = col_tile(bng[l, si], KX, tag="ffcol")
            bnb_col = col_tile(bnb[l, si], KX, tag="ffcol")
            rsq = float(1.0 / np.sqrt(1.0 + BN_EPS))
            A_col = wcol.tile([128, KX], dt, tag="ffcol")
            sc.activation(A_col, bng_col, AF.Copy, scale=rsq)
            B_col = wcol.tile([128, KX], dt, tag="ffcol")
            v.tensor_tensor(out=B_col, in0=cb_col, in1=A_col, op=ALU.mult)
            v.tensor_tensor(out=B_col, in0=B_col, in1=bnb_col, op=ALU.add)
            g_col = col_tile(lng[l, 3 if s == "b" else 4], KE, tag="lncol")
            bb_col = col_tile(lnb[l, 3 if s == "b" else 4], KE, tag="lncol")

            hts = [None] * NC
            xts = [None] * NC
            hl0 = [None] * NC   # last col scaled by w0
            hf2 = [None] * NC   # first col scaled by w2

            def compute_h(ci):
                xt = load_x_chunk(src, ci)
                xts[ci] = xt
                ht = pb.tile([128, KX, C], dt, tag="a16")
                for g in range(2):
                    phs = [ps.tile([128, C], F32, tag="mm", name=f"ph{_i}") for _i in range(4)]
                    for k in range(KE):
                        for j in range(4):
                            m = g * 4 + j
                            mm(phs[j], w1t[:, k, m * 128:(m + 1) * 128],
                               xt[:, k, :], start=(k == 0),
                               stop=(k == KE - 1))
                    for j in range(4):
                        m = g * 4 + j
                        sc.activation(ht[:, m, :], phs[j], AF.Relu,
                                      bias=b1_col[:, m:m + 1])
                        gp.tensor_scalar_min(ht[:, m, :], ht[:, m, :], 6.0)
                hts[ci] = ht
                l0 = phl.tile([128, KX, 1], dt, tag="hl")
                f2 = phl.tile([128, KX, 1], dt, tag="hf")
                for m in range(KX):
                    sc.activation(l0[:, m, :], ht[:, m, C - 1:C], AF.Copy,
                                  scale=w0_col[:, m:m + 1])
                    sc.activation(f2[:, m, :], ht[:, m, 0:1], AF.Copy,
                                  scale=w2_col[:, m:m + 1])
                hl0[ci], hf2[ci] = l0, f2

            def conv_tail(ci):
                ht = hts[ci]
                h2 = pb.tile([128, KX, C], MMDT, tag="a16")
                for m in range(KX):
                    acc = pc.tile([128, C], dt, tag="a2")
                    tmp = pc.tile([128, C], dt, tag="a2")
                    sc.activation(acc, ht[:, m, :], AF.Copy,
                                  scale=w1c_col[:, m:m + 1])
                    sc.activation(tmp, ht[:, m, :], AF.Copy,
                                  scale=w0_col[:, m:m + 1])
                    gp.tensor_tensor(out=acc[:, 1:C], in0=acc[:, 1:C],
                                     in1=tmp[:, 0:C - 1], op=ALU.add)
                    if ci > 0:
                        gp.tensor_tensor(out=acc[:, 0:1], in0=acc[:, 0:1],
                                         in1=hl0[ci - 1][:, m, :], op=ALU.add)
                    sc.activation(tmp, ht[:, m, :], AF.Copy,
                                  scale=w2_col[:, m:m + 1])
                    gp.tensor_tensor(out=acc[:, 0:C - 1], in0=acc[:, 0:C - 1],
                                     in1=tmp[:, 1:C], op=ALU.add)
                    if ci < NC - 1:
                        gp.tensor_tensor(out=acc[:, C - 1:C],
                                         in0=acc[:, C - 1:C],
                                         in1=hf2[ci + 1][:, m, :], op=ALU.add)
                    acc2 = pc.tile([128, C], dt, tag="a2")
                    sc.activation(acc2, acc, AF.Relu,
                                  scale=A_col[:, m:m + 1],
                                  bias=B_col[:, m:m + 1])
                    v.tensor_scalar_min(h2[:, m, :], acc2, 6.0)
                rt = pa.tile([128, KE, C], MMDT, tag="a8")
                pws = [ps.tile([128, C], F32, tag="mm", name=f"pw{_i}") for _i in range(KE)]
                for k in range(KX):
                    for m in range(KE):
                        mm(pws[m], w2t[:, k, m * 128:(m + 1) * 128],
                           h2[:, k, :], start=(k == 0), stop=(k == KX - 1))
                for m in range(KE):
                    rtf = pc.tile([128, C], dt, tag="a2")
                    sc.activation(rtf, pws[m], AF.Identity,
                                  bias=b2_col[:, m:m + 1])
                    v.tensor_tensor(out=rt[:, m, :], in0=rtf,
                                    in1=F(xts[ci][:, m, :]), op=ALU.add)
                outt = pa.tile([128, KE, C], dt, tag="a8")
                ln_stats_apply([rt[:, m, :] for m in range(KE)], g_col, bb_col,
                               [outt[:, m, :] for m in range(KE)])
                store_chunk(dst, ci, outt)
                hts[ci] = xts[ci] = None

            compute_h(0)
            for ci in range(1, NC):
                compute_h(ci)
                conv_tail(ci - 1)
            conv_tail(NC - 1)

        # ---- layers ----
        for l in range(L):
            bsrc = rs["b", 0] if l == 0 else rs["b", (l - 1, 3)]
            lsrc = rs["l", 0] if l == 0 else rs["l", (l - 1, 3)]
            attn(l, 0, bsrc, bsrc, make_self_tail(l, "b", rs["b", (l, 1)]))
            attn(l, 1, lsrc, lsrc, make_self_tail(l, "l", rs["l", (l, 1)]))
            attn(l, 2, rs["b", (l, 1)], rs["l", (l, 1)],
                 make_cross_tail(l, "b", rs["b", (l, 2)]))
            attn(l, 3, rs["l", (l, 1)], rs["b", (l, 1)],
                 make_cross_tail(l, "l", rs["l", (l, 2)]))
            ffn(l, "b", rs["b", (l, 2)], rs["b", (l, 3)])
            ffn(l, "l", rs["l", (l, 2)], rs["l", (l, 3)])

        PHASES.append(("final", len(nc.inst_map)))
        # ---- final head ----
        fw1t = wbig.tile([128, 2 * KE, E2], MMDT, tag="w")
        nc.sync.dma_start(out=fw1t,
                          in_=M(fw1.rearrange("(k p) g -> p k g", p=128)))
        fw2t = wsm.tile([128, 2, E], MMDT, tag="ws")
        nc.sync.dma_start(out=fw2t,
                          in_=M(fw2.rearrange("(k p) e -> p k e", p=128)))
        rw1t = wsm.tile([128, KE, E4], MMDT, tag="ws")
        nc.sync.dma_start(out=rw1t,
                          in_=M(rw1.rearrange("(k p) g -> p k g", p=128)))
        rw2t = wrow.tile([128, E8], MMDT, tag="row")
        nc.sync.dma_start(out=rw2t, in_=M(rw2))
        rw3t = wrow.tile([E8, 16], MMDT, tag="row")
        v.memset(F(rw3t), 0.0)
        nc.sync.dma_start(out=rw3t[:, 0:c.OUT], in_=M(rw3))
        rb3_row = wrow.tile([1, 16], MMDT, tag="row")
        v.memset(F(rb3_row), 0.0)
        nc.sync.dma_start(out=rb3_row[:, 0:c.OUT], in_=M(rb3[None, :]))
        fb1_col = col_tile(fb1, 2, tag="fcol")
        fb2_col = col_tile(fb2, KE, tag="fcol")
        flng_col = col_tile(flng, KE, tag="fcol")
        flnb_col = col_tile(flnb, KE, tag="fcol")
        rb1_col = col_tile(rb1, 1, tag="fcol")
        rb2_col = wcol.tile([E8, 1], dt, tag="fcol")
        nc.sync.dma_start(out=rb2_col, in_=rb2[:, None])
        out_ap = out_dram.ap()

        bsrc, lsrc = rs["b", (L - 1, 3)], rs["l", (L - 1, 3)]
        for ci in range(NC):
            xb = load_x_chunk(bsrc, ci)
            xl = load_x_chunk(lsrc, ci)
            f1t = [pc.tile([128, C], MMDT, tag="a2", name=f"f1t{_i}")
                   for _i in range(2)]
            pfs = [ps.tile([128, C], F32, tag="mm", name=f"pf{_i}") for _i in range(2)]
            for k in range(2 * KE):
                rhs = xb[:, k, :] if k < KE else xl[:, k - KE, :]
                for m in range(2):
                    mm(pfs[m], fw1t[:, k, m * 128:(m + 1) * 128], rhs,
                       start=(k == 0), stop=(k == 2 * KE - 1))
            for m in range(2):
                f1f = pc.tile([128, C], dt, tag="a2")
                sc.activation(f1f, pfs[m], AF.Relu, bias=fb1_col[:, m:m + 1])
                v.tensor_scalar_min(f1t[m], f1f, 6.0)
            ft = pa.tile([128, KE, C], MMDT, tag="a8")
            pf2s = [ps.tile([128, C], F32, tag="mm", name=f"pf2{_i}") for _i in range(KE)]
            for k in range(2):
                for m in range(KE):
                    mm(pf2s[m], fw2t[:, k, m * 128:(m + 1) * 128],
                       f1t[k], start=(k == 0), stop=(k == 1))
            for m in range(KE):
                v.tensor_scalar_add(ft[:, m, :], pf2s[m], fb2_col[:, m:m + 1])
            frt = pa.tile([128, KE, C], MMDT, tag="a8")
            ln_stats_apply([ft[:, m, :] for m in range(KE)], flng_col,
                           flnb_col, [frt[:, m, :] for m in range(KE)],
                           relu=True, out_f32r=True)
            p1 = ps.tile([128, C], F32, tag="mm")
            for k in range(KE):
                mm(p1, rw1t[:, k, :], frt[:, k, :], start=(k == 0),
                   stop=(k == KE - 1))
            h1f = pc.tile([128, C], dt, tag="a2")
            sc.activation(h1f, p1, AF.Relu, bias=rb1_col[:, 0:1])
            h1t = pc.tile([128, C], MMDT, tag="a2")
            v.tensor_scalar_min(h1t, h1f, 6.0)
            p2 = ps.tile([E8, C], F32, tag="mm")
            mm(p2, rw2t, h1t, start=True, stop=True)
            h2f = pc.tile([E8, C], dt, tag="a2")
            sc.activation(h2f, p2, AF.Relu, bias=rb2_col[:, 0:1])
            h2t = pc.tile([E8, C], MMDT, tag="a2")
            v.tensor_scalar_min(h2t, h2f, 6.0)
            ot = pc.tile([128, NTT, c.OUT], dt, tag="a2")
            for tt in range(NTT):
                p3 = ps.tile([128, 16], F32, tag="mm")
                mm(p3, h2t[:, tt * 128:(tt + 1) * 128], rw3t,
                   start=True, stop=False)
                mm(p3, ONES_ROW[:, 0:128], rb3_row, start=False, stop=True)
                sc.activation(ot[:, tt, :], p3[:, 0:c.OUT], AF.Copy)
            nc.sync.dma_start(
                out=out_ap[ci * C:(ci + 1) * C, :].rearrange(
                    "(tt p) o -> p tt o", p=128),
                in_=ot)

    return din, out_dram


# ======================================================================
# kernel() entry point: full inputs in, full outputs out (8-core SPMD).
# ======================================================================
import concourse.bacc as _bacc
from concourse.bass_utils import run_bass_kernel_spmd as _run_spmd

_N_CORES = 8
_CACHE = {}


def _get_nc():
    if "nc" not in _CACHE:
        nc = _bacc.Bacc("TRN2", target_bir_lowering=False, debug=False)
        build(nc, Cfg())
        nc.finalize()
        _CACHE["nc"] = nc
    return _CACHE["nc"]


def kernel(**inputs):
    nc = _get_nc()
    cfg = Cfg()
    consts = host_constants(cfg)
    arr = {k: np.ascontiguousarray(np.asarray(v, dtype=np.float32))
           for k, v in inputs.items()}
    shared = {k: a for k, a in arr.items()
              if k not in ("body_feats", "limb_feats")}
    shared.update(consts)
    in_maps = []
    for i in range(_N_CORES):
        m = dict(shared)
        m["body_feats"] = np.ascontiguousarray(arr["body_feats"][i])
        m["limb_feats"] = np.ascontiguousarray(arr["limb_feats"][i])
        in_maps.append(m)
    res = run_kernel_spmd_cached(nc, in_maps)
    out = np.stack([res[i]["out"] for i in range(_N_CORES)], axis=0)
    return out.astype(np.float32)


def run_kernel_spmd_cached(nc, in_maps, **kw):
    r = _run_spmd(nc, in_maps, list(range(_N_CORES)), **kw)
    _CACHE["last_result"] = r
    return r.results

